# revision 1
# baseline (speedup 1.0000x reference)
"""Longformer self-attention Trainium2 kernel (8-core SPMD).

Sharding: core c handles batch b = c//4 and heads [3*(c%4), 3*(c%4)+3).
Each core receives pre-sliced/augmented inputs and computes [4096, 192]
(its 3 heads' output dims); the host reassembles [2, 4096, 768].

Device-side math per core (heads h in 0..3, all layouts chosen so no
on-device transposes are needed):
  - xT [768, 4096] = hidden[b].T; q-scale folded into Wq/Wqg on host.
  - qT/kT/kgT produced transposed [64, S] (W stationary), v/vg produced
    natural [S, 64] with a ones column appended (xT chunks stationary);
    biases are added during the PSUM->SBUF evacuation on DVE.
  - Band scores computed transposed: sT[kpos, q] per 256-query block over
    a 768-wide kpos window, as 6 [128, 256] matmuls.
  - exp() without max subtraction (logits are O(0.3): x ~ N(0,1),
    W ~ 0.02 N(0,1), so exp is numerically safe); band-validity and
    global-exclusion masks are applied multiplicatively (0/1 bf16 masks
    on DVE) after the exp — equivalent to the reference's -inf / -10000
    additive logits, whose softmax contributions underflow to exactly 0.
  - PV: attn[q, 0:64] and the softmax denominator (ones column of v) come
    out of one accumulated PSUM [128, 65]; normalize = reciprocal + mul.
  - Global-token rows (0..15) use the qg/kg/vg projections with the same
    transposed-score trick and overwrite rows 0..15 of block 0.
"""

import sys

sys.path.insert(0, "/opt/trn_rl_repo")

import numpy as np
import ml_dtypes

B, S, Dm, H, WIN, G, HD = 2, 4096, 768, 12, 256, 16, 64
HPC = 3            # heads per core
NCORES = 8
DPC = HPC * HD     # 192 output dims per core
NB = S // WIN      # 16 query blocks
NKC = S // 128     # 32 kpos chunks of 128
SCALE = 1.0 / 8.0  # 1/sqrt(64)

_CACHE = {}


def _mask_classes():
    """Multiplicative {0,1} masks in transposed-score orientation
    [kpos_local p, q_local r], applied to exp(scores).

    Chunk c of block t covers kpos = (2t-2+c)*128 + p, query i = 256t + r.
    Keep (1.0) iff the slot is band-valid and not a global key; global-key
    slots (kpos < G) and out-of-band slots contribute exactly 0 to the
    reference softmax (exp(-inf) / exp(x - 10000) both underflow to 0).
    """
    def build(t, c):
        p = np.arange(128)[:, None]
        r = np.arange(256)[None, :]
        kpos = (2 * t - 2 + c) * 128 + p
        i = 256 * t + r
        keep = (np.abs(kpos - i) <= WIN) & (kpos >= 0) & (kpos < S) & (kpos >= G)
        return keep.astype(np.float32)

    classes = {
        "t0c2": build(0, 2),
        "t1c0": build(1, 0),
        "c0": build(7, 0),
        "c1": build(7, 1),
        "c4": build(7, 4),
        "c5": build(7, 5),
    }
    lookup = {}
    for t in range(NB):
        cl, ch = _chunk_range(t)
        for c in range(cl, ch):
            if t == 0 and c == 2:
                mi = "t0c2"
            elif t == 1 and c == 0:
                mi = "t1c0"
            elif c == 0:
                mi = "c0"
            elif c == 1:
                mi = "c1"
            elif c == 4:
                mi = "c4"
            elif c == 5:
                mi = "c5"
            else:
                mi = None
            if mi is not None:
                assert np.array_equal(classes[mi], build(t, c)), (t, c, mi)
            else:
                assert np.all(build(t, c) == 1.0), (t, c)
            lookup[(t, c)] = mi
    return classes, lookup


def _chunk_range(t):
    if t == 0:
        return 2, 6
    if t == NB - 1:
        return 0, 4
    return 0, 6


def _patch_drain_and_barrier():
    """The walrus build in this container rejects >1 sync-wait on the CTRL
    (Drain) instruction that TileContext emits at exit ("Too many sync wait
    commands"). Split the waits: keep one on the drain, emit the rest as
    explicit single-sem wait_ge instructions on the sync engine before the
    barrier. Semantics preserved: all sems still quiesce before the
    sem-clear + barrier."""
    import concourse.tile as tile
    from concourse import mybir
    from concourse.vector_clock import ScopedClock

    if getattr(tile.TileContext, "_ant_drain_patch", False):
        return

    def _drain_and_barrier(self, tick_clock, wait_clock):
        nc = self.nc
        drain_inst = nc.sync.drain()
        wait_clock.add_sem_waits(
            drain_inst.ins, ScopedClock({None: tick_clock.global_clock})
        )
        si = drain_inst.ins.sync_info
        waits = list(si.on_wait) if si is not None else []
        if len(waits) > 1:
            drain_inst.ins.sync_info = mybir.SyncInfo(
                on_wait=[waits[0]], on_update=list(si.on_update)
            )
            allocated = self.sems.allocated()
            by_name = {}
            for key, sem in allocated.items():
                by_name[str(key)] = sem
                nm = getattr(sem, "name", None)
                if nm is not None:
                    by_name[str(nm)] = sem
            for w in waits[1:]:
                sem = by_name[w.ant_name]
                nc.sync.wait_ge(sem, w.wait_value)
        nc.all_engine_barrier()
        assert self.sems is not None
        popped = nc._tile_sem_poison_stack.pop()
        assert popped is self._sem_poison
        nc.clear_and_free_semaphores(list(self.sems.allocated().values()))
        nc.all_engine_barrier()

    tile.TileContext._drain_and_barrier = _drain_and_barrier
    tile.TileContext._ant_drain_patch = True


def _build_program():
    import concourse.bass as bass
    import concourse.tile as tile
    from concourse import bacc, mybir

    _patch_drain_and_barrier()

    f32 = mybir.dt.float32
    f32r = mybir.dt.float32r
    bf16 = mybir.dt.bfloat16
    AF = mybir.ActivationFunctionType
    ALU = mybir.AluOpType

    # Bacc (not plain Bass): its compile() pipeline runs
    # generate_event_semaphores, which splits multi-sem waits — this
    # walrus build allows at most one sync wait per instruction.
    nc = bacc.Bacc(None)

    xT = nc.dram_tensor("xT", [Dm, S], bf16, kind="ExternalInput")
    Wq = nc.dram_tensor("Wq", [Dm, DPC], bf16, kind="ExternalInput")
    Wk = nc.dram_tensor("Wk", [Dm, DPC], bf16, kind="ExternalInput")
    Wkg = nc.dram_tensor("Wkg", [Dm, DPC], bf16, kind="ExternalInput")
    Wqg = nc.dram_tensor("Wqg", [Dm, DPC], bf16, kind="ExternalInput")
    Wvvg = nc.dram_tensor("Wvvg", [Dm, 2 * DPC], bf16, kind="ExternalInput")
    # per-head bias columns [64, 3]: column h = bias slice for head h
    b_q = nc.dram_tensor("b_q", [HD, HPC], f32, kind="ExternalInput")
    b_k = nc.dram_tensor("b_k", [HD, HPC], f32, kind="ExternalInput")
    b_kg = nc.dram_tensor("b_kg", [HD, HPC], f32, kind="ExternalInput")
    b_qg = nc.dram_tensor("b_qg", [HD, HPC], f32, kind="ExternalInput")
    # broadcast v/vg bias: [128 partitions, head, (v|vg), 64]
    b_vvg = nc.dram_tensor("b_vvg", [128, HPC, 2, HD], f32, kind="ExternalInput")
    out_d = nc.dram_tensor("out", [S, DPC], f32, kind="ExternalOutput")

    classes, lookup = _mask_classes()
    mask_names = list(classes.keys())
    mask_np = np.stack([classes[k] for k in mask_names], axis=1)  # [128, 6, 256]
    masks_d = nc.inline_tensor(mask_np.astype(ml_dtypes.bfloat16), name="masks")
    midx = {k: i for i, k in enumerate(mask_names)}

    from contextlib import ExitStack

    with tile.TileContext(nc) as tc, ExitStack() as ctx:
        const = ctx.enter_context(tc.tile_pool(name="const", bufs=1))
        ph = ctx.enter_context(tc.tile_pool(name="ph", bufs=1))
        xpool = ctx.enter_context(tc.tile_pool(name="xpool", bufs=3))
        bx = ctx.enter_context(tc.tile_pool(name="bx", bufs=3))
        sbS = ctx.enter_context(tc.tile_pool(name="sbS", bufs=6))
        psA = ctx.enter_context(tc.tile_pool(name="psA", bufs=2, space="PSUM"))
        psB = ctx.enter_context(tc.tile_pool(name="psB", bufs=2, space="PSUM"))

        # issue exactly the first projection group's operands first (Wq,
        # x-tile 0), then everything else — minimizes the startup PE stall
        w6 = {}
        w6["q"] = const.tile([128, 6, DPC], bf16, tag="w6q", name="w6q")
        nc.sync.dma_start(
            out=w6["q"], in_=Wq[:, :].rearrange("(c p) d -> p c d", p=128)
        )
        xt0 = xpool.tile([128, 6, 512], bf16, tag="xt", name="xt")
        nc.sync.dma_start(
            out=xt0, in_=xT[:, 0:512].rearrange("(c p) s -> p c s", p=128)
        )

        # ---- remaining constants to SBUF ----
        for nm, dram, width in (
            ("k", Wk, DPC),
            ("kg", Wkg, DPC),
            ("qg", Wqg, DPC),
            ("vvg", Wvvg, 2 * DPC),
        ):
            w6[nm] = const.tile([128, 6, width], bf16, tag=f"w6{nm}", name=f"w6{nm}")
            nc.sync.dma_start(
                out=w6[nm], in_=dram[:, :].rearrange("(c p) d -> p c d", p=128)
            )
        bias = {}
        for nm, dram in (("q", b_q), ("k", b_k), ("kg", b_kg), ("qg", b_qg)):
            bias[nm] = const.tile([HD, HPC], f32, tag=f"b{nm}", name=f"b{nm}")
            nc.sync.dma_start(out=bias[nm], in_=dram[:])
        bvvg_sb = const.tile([128, HPC, 2, HD], f32, tag="bvvg", name="bvvg_sb")
        nc.sync.dma_start(out=bvvg_sb, in_=b_vvg[:])
        masks_sb = const.tile([128, 6, 256], bf16, tag="masks", name="masks_sb")
        nc.sync.dma_start(out=masks_sb, in_=masks_d[:])

        # ---- persistent per-head tensors ----
        qT = [ph.tile([64, S], bf16, tag=f"qT{h}", name=f"qT{h}") for h in range(HPC)]
        kT = [ph.tile([64, S], bf16, tag=f"kT{h}", name=f"kT{h}") for h in range(HPC)]
        kgT = [ph.tile([64, S], bf16, tag=f"kgT{h}", name=f"kgT{h}") for h in range(HPC)]
        # v/vg interleaved with ones column: [:, chunk, 2h+0, :] = v head h,
        # [:, chunk, 2h+1, :] = vg head h ([:, :, :, 64] = 1.0)
        vall = ph.tile([128, NKC, 2 * HPC, HD + 1], bf16, tag="vall", name="vall")
        nc.vector.memset(vall[:, :, :, HD : HD + 1], 1.0)
        selexp = [
            ph.tile([G, S], bf16, tag=f"selexp{h}", name=f"selexp{h}")
            for h in range(HPC)
        ]
        qgT = [ph.tile([64, G], bf16, tag=f"qgT{h}", name=f"qgT{h}") for h in range(HPC)]
        eg = [
            ph.tile([128, NKC, G], bf16, tag=f"eg{h}", name=f"eg{h}")
            for h in range(HPC)
        ]
        outg = [ph.tile([G, HD], f32, tag=f"outg{h}", name=f"outg{h}") for h in range(HPC)]

        def mm(out, lhsT, rhs, start, stop):
            nc.tensor.matmul(out, lhsT, rhs, start=start, stop=stop)

        AFexp = AF.Exp

        # ---- projections ----
        for st in range(8):
            ssl = slice(512 * st, 512 * (st + 1))
            if st == 0:
                xt = xt0
            else:
                xt = xpool.tile([128, 6, 512], bf16, tag="xt", name="xt")
                nc.sync.dma_start(
                    out=xt, in_=xT[:, ssl].rearrange("(c p) s -> p c s", p=128)
                )

            # q/k/kg: transposed layout, W stationary
            for nm in ("q", "k", "kg"):
                dstt = {"q": qT, "k": kT, "kg": kgT}[nm]
                for dc, (d0, d1) in enumerate(((0, 128), (128, 192))):
                    ps = psB.tile([d1 - d0, 512], f32, tag="small", name="psqk")
                    for kc in range(6):
                        mm(ps, w6[nm][:, kc, d0:d1], xt[:, kc, :], kc == 0, kc == 5)
                    # evacuate per head rows, adding bias
                    for h in range(HPC):
                        r0 = max(d0, h * HD)
                        r1 = min(d1, (h + 1) * HD)
                        if r0 >= r1:
                            continue
                        nc.vector.tensor_scalar_add(
                            dstt[h][r0 - h * HD : r1 - h * HD, ssl],
                            ps[r0 - d0 : r1 - d0, :],
                            bias[nm][r0 - h * HD : r1 - h * HD, h : h + 1],
                        )

            # v/vg: natural layout, xT chunks stationary
            for sc in range(4):
                ci = 4 * st + sc
                msl = slice(128 * sc, 128 * (sc + 1))
                psv = psB.tile([128, 2 * DPC], f32, tag="small", name="psv")
                for kc in range(6):
                    mm(psv, xt[:, kc, msl], w6["vvg"][:, kc, :], kc == 0, kc == 5)
                # one strided op: psv[:, gi*192 + h*64 + d] -> vall[:, ci, 2h+gi, d]
                src = bass.AP(
                    tensor=psv.tensor,
                    offset=psv.offset,
                    ap=[psv.ap[0], [HD, HPC], [DPC, 2], [1, HD]],
                )
                dst = vall[:, ci, :, 0:HD].rearrange("p (h g) d -> p h g d", h=HPC)
                nc.vector.tensor_add(dst, src, bvvg_sb)

            # global columns for this s-tile: sel = q . k[:G], exp
            # (rides the warm projection phase; kT[:, :G] exists once st >= 1)
            for h in range(HPC):
                if st == 0:
                    continue
                sps = psB.tile([G, 512], f32, tag="small", name="sps")
                mm(sps, kT[h][:, 0:G], qT[h][:, ssl], True, True)
                nc.scalar.activation(out=selexp[h][:, ssl], in_=sps, func=AFexp)

            if st == 0:
                # qg: [64, 16] per head, transposed
                for h in range(HPC):
                    psq = psB.tile([64, G], f32, tag="small", name="psqg")
                    for kc in range(6):
                        mm(
                            psq,
                            w6["qg"][:, kc, HD * h : HD * (h + 1)],
                            xt[:, kc, 0:G],
                            kc == 0,
                            kc == 5,
                        )
                    nc.vector.tensor_scalar_add(
                        qgT[h], psq, bias["qg"][:, h : h + 1]
                    )

        AFexp = AF.Exp

        # ---- global-token rows: full attention with qg/kg/vg ----
        for h in range(HPC):
            gps = psB.tile([128, NKC, G], f32, tag="small", name="gps")
            for c in range(NKC):
                mm(gps[:, c, :], kgT[h][:, 128 * c : 128 * (c + 1)], qgT[h], True, True)
            nc.scalar.activation(out=eg[h], in_=gps, func=AFexp)
            ops = psB.tile([G, HD + 1], f32, tag="small", name="ops")
            for c in range(NKC):
                mm(ops, eg[h][:, c, :], vall[:, c, 2 * h + 1, :], c == 0, c == NKC - 1)
            recg = sbS.tile([G, 1], f32, tag="recg", name="recg")
            nc.vector.reciprocal(recg, ops[:, HD : HD + 1])
            nc.vector.tensor_scalar_mul(outg[h], ops[:, 0:HD], recg)

        # sel for s-tile 0 (kT[:, :G] only ready after s-tile 0 projections)
        for h in range(HPC):
            sps = psB.tile([G, 512], f32, tag="small", name="sps")
            mm(sps, kT[h][:, 0:G], qT[h][:, 0:512], True, True)
            nc.scalar.activation(out=selexp[h][:, 0:512], in_=sps, func=AFexp)

        # ---- banded local attention ----
        # t-major: the three heads' blocks interleave, keeping the PE fed
        # while one head's exp/mask sits between QK and PV (HAM stays warm)
        for t in range(NB):
            for h in range(HPC):
                cl, ch = _chunk_range(t)
                qsl = slice(256 * t, 256 * (t + 1))
                sc_ps = psA.tile([128, 6, 256], f32, tag="scores", name="sc_ps")
                for c in range(cl, ch):
                    j = 2 * t - 2 + c
                    mm(
                        sc_ps[:, c, :],
                        kT[h][:, 128 * j : 128 * (j + 1)],
                        qT[h][:, qsl],
                        True,
                        True,
                    )
                bexp = bx.tile([128, 6, 256], bf16, tag="bexp", name="bexp")
                nc.scalar.activation(
                    out=bexp[:, cl:ch, :], in_=sc_ps[:, cl:ch, :], func=AFexp
                )
                for c in range(cl, ch):
                    mi = lookup[(t, c)]
                    if mi is not None:
                        nc.vector.tensor_mul(
                            bexp[:, c, :], bexp[:, c, :], masks_sb[:, midx[mi], :]
                        )
                for half in range(2):
                    q0 = 256 * t + 128 * half
                    hs = slice(128 * half, 128 * (half + 1))
                    at = psB.tile([128, HD + 1], f32, tag="small", name="at")
                    for c in range(cl, ch):
                        j = 2 * t - 2 + c
                        mm(at, bexp[:, c, hs], vall[:, j, 2 * h, :], c == cl, False)
                    mm(at, selexp[h][:, q0 : q0 + 128], vall[0:G, 0, 2 * h, :], False, True)
                    rec = sbS.tile([128, 1], f32, tag="rec", name="rec")
                    nc.vector.reciprocal(rec, at[:, HD : HD + 1])
                    osb = sbS.tile([128, HD], f32, tag="osb", name="osb")
                    nc.vector.tensor_scalar_mul(osb, at[:, 0:HD], rec)
                    if t == 0 and half == 0:
                        nc.vector.tensor_copy(out=osb[0:G, :], in_=outg[h])
                    nc.sync.dma_start(
                        out=out_d[q0 : q0 + 128, HD * h : HD * (h + 1)], in_=osb
                    )

    return nc


def _get_program():
    if "nc" not in _CACHE:
        nc = _build_program()
        nc.finalize()
        _CACHE["nc"] = nc
    return _CACHE["nc"]


def _prep_in_maps(hidden_states, Wq, bq, Wk, bk, Wv, bv, Wqg, bqg, Wkg, bkg, Wvg, bvg):
    hs = np.asarray(hidden_states, dtype=np.float32)
    f32 = np.float32
    in_maps = []
    for c in range(NCORES):
        b = c // 4
        cols = slice(HD * 3 * (c % 4), HD * (3 * (c % 4) + 3))

        def bcol(v, scale=1.0):
            # [192] -> [64, 3] column-per-head
            return np.ascontiguousarray(
                (np.asarray(v)[cols] * scale).reshape(HPC, HD).T.astype(f32)
            )

        bvvg = np.stack(
            [
                np.asarray(bv)[cols].reshape(HPC, HD),
                np.asarray(bvg)[cols].reshape(HPC, HD),
            ],
            axis=1,
        ).astype(f32)  # [3, 2, 64]
        in_maps.append(
            {
                "xT": np.ascontiguousarray(hs[b].T).astype(ml_dtypes.bfloat16),
                "Wq": np.ascontiguousarray(np.asarray(Wq)[:, cols] * SCALE).astype(ml_dtypes.bfloat16),
                "Wk": np.ascontiguousarray(np.asarray(Wk)[:, cols]).astype(ml_dtypes.bfloat16),
                "Wkg": np.ascontiguousarray(np.asarray(Wkg)[:, cols]).astype(ml_dtypes.bfloat16),
                "Wqg": np.ascontiguousarray(np.asarray(Wqg)[:, cols] * SCALE).astype(ml_dtypes.bfloat16),
                "Wvvg": np.concatenate(
                    [np.asarray(Wv)[:, cols], np.asarray(Wvg)[:, cols]], axis=1
                ).astype(ml_dtypes.bfloat16),
                "b_q": bcol(bq, SCALE),
                "b_k": bcol(bk),
                "b_kg": bcol(bkg),
                "b_qg": bcol(bqg, SCALE),
                "b_vvg": np.ascontiguousarray(
                    np.broadcast_to(bvvg[None], (128, HPC, 2, HD))
                ),
            }
        )
    return in_maps


def kernel(
    hidden_states,
    Wq,
    bq,
    Wk,
    bk,
    Wv,
    bv,
    Wqg,
    bqg,
    Wkg,
    bkg,
    Wvg,
    bvg,
    n_global,
):
    from concourse.bass_utils import run_bass_kernel_spmd

    assert int(n_global) == G
    nc = _get_program()
    in_maps = _prep_in_maps(
        hidden_states, Wq, bq, Wk, bk, Wv, bv, Wqg, bqg, Wkg, bkg, Wvg, bvg
    )
    res = run_bass_kernel_spmd(nc, in_maps, list(range(NCORES)))
    out = np.zeros((B, S, Dm), np.float32)
    for c in range(NCORES):
        b = c // 4
        cols = slice(HD * 3 * (c % 4), HD * (3 * (c % 4) + 3))
        out[b, :, cols] = res.results[c]["out"]
    return out



# revision 13
# speedup vs baseline: 1.1512x; 1.1512x over previous
"""Longformer self-attention Trainium2 kernel (8-core SPMD), fused-phase v2.

Sharding: core c handles batch b = c//4 and heads [3*(c%4), 3*(c%4)+3).
Each core computes [4096, 192] (its 3 heads' output dims); the host
reassembles [2, 4096, 768].

v2 design (vs v1 two-phase):
  - Wq|Wk|Wkg merged into one [768, 576] weight -> 5 full-width PSUM
    d-chunks per s-tile (4x M=128 + 1x M=64) instead of 6 (3 proj x
    (M=128 + M=64)): fewer N=512 matmuls at full array width.
  - Projections and banded attention are FUSED: at s-tile st we emit
    the st projections, QK for blocks t=2st-2, 2st-1 (whose k-window
    just completed), and PV for the previous s-tile's blocks. The Act
    exp of a block runs while the PE streams the next s-tile's
    projections, so the in-order PE queue never waits on exp/mask.
  - Global-row scores (glT chunks) are computed per s-tile as the kg
    projection lands; global PV + normalize run in a short tail.
  - Band masks applied as at most 2 wide DVE multiplies per block
    from 4 precomputed mask classes (t0/t1/mid/t15) [128, 6, 256].
  - Block 0's half-0 output DMA skips rows 0..15; the global-row
    result is DMA'd to out[0:16] separately (disjoint -> no race).

Math identical to v1: exp without max subtraction (logits are O(1)),
multiplicative 0/1 masks after exp, softmax denominator via a ones
column appended to v/vg, bias adds during PSUM evacuation.
"""

import sys

sys.path.insert(0, "/opt/trn_rl_repo")

import numpy as np
import ml_dtypes

B, S, Dm, H, WIN, G, HD = 2, 4096, 768, 12, 256, 16, 64
HPC = 3            # heads per core
NCORES = 8
DPC = HPC * HD     # 192 output dims per core
NB = S // WIN      # 16 query blocks
NKC = S // 128     # 32 kpos chunks of 128
NST = 8            # s-tiles of 512
SCALE = 1.0 / 8.0  # 1/sqrt(64)
QKK = 3 * DPC      # 576 merged q|k|kg output cols

_CACHE = {}


def _chunk_range(t):
    if t == 0:
        return 2, 6
    if t == NB - 1:
        return 0, 4
    return 0, 6


def _build_mask(t, c):
    p = np.arange(128)[:, None]
    r = np.arange(256)[None, :]
    kpos = (2 * t - 2 + c) * 128 + p
    i = 256 * t + r
    keep = (np.abs(kpos - i) <= WIN) & (kpos >= 0) & (kpos < S) & (kpos >= G)
    return keep.astype(np.float32)


def _mask_classes():
    """Packed mask chunks [nmask, 128, 256] and, per block t, the list of
    (chunk_lo, chunk_hi, packed_idx) mask ops: multiply bexp[:, lo:hi, :]
    by packed[idx : idx + (hi-lo)].  Only non-trivial chunks are stored;
    each block needs at most 2 ops on contiguous chunk/packed ranges."""
    packed = []
    pidx = {}  # (cls, chunk) -> packed index

    def cls_of(t):
        return 0 if t == 0 else (1 if t == 1 else (3 if t == NB - 1 else 2))

    ops = {}
    for cls, trep in ((0, 0), (1, 1), (2, 7), (3, NB - 1)):
        cl, ch = _chunk_range(trep)
        need = [
            c for c in range(cl, ch) if not np.all(_build_mask(trep, c) == 1.0)
        ]
        # group consecutive chunks; store each group contiguously in packed
        groups = []
        for c in need:
            if groups and groups[-1][-1] == c - 1:
                groups[-1].append(c)
            else:
                groups.append([c])
        clsops = []
        for grp in groups:
            base = len(packed)
            for c in grp:
                pidx[(cls, c)] = len(packed)
                packed.append(_build_mask(trep, c))
            clsops.append((grp[0], grp[-1] + 1, base))
        ops[cls] = clsops
    # verify every block against the reference builder
    for t in range(NB):
        cls = cls_of(t)
        cl, ch = _chunk_range(t)
        covered = set()
        for lo, hi, base in ops[cls]:
            for c in range(lo, hi):
                assert np.array_equal(packed[base + (c - lo)], _build_mask(t, c))
                covered.add(c)
        for c in range(cl, ch):
            if c not in covered:
                assert np.all(_build_mask(t, c) == 1.0), (t, c)
    return np.stack(packed), ops, cls_of


def _patch_drain_and_barrier():
    """The walrus build in this container rejects >1 sync-wait on the CTRL
    (Drain) instruction that TileContext emits at exit ("Too many sync wait
    commands"). Split the waits: keep one on the drain, emit the rest as
    explicit single-sem wait_ge instructions on the sync engine before the
    barrier. Semantics preserved: all sems still quiesce before the
    sem-clear + barrier."""
    import concourse.tile as tile
    from concourse import mybir
    from concourse.vector_clock import ScopedClock

    if getattr(tile.TileContext, "_ant_drain_patch", False):
        return

    def _drain_and_barrier(self, tick_clock, wait_clock):
        nc = self.nc
        drain_inst = nc.sync.drain()
        wait_clock.add_sem_waits(
            drain_inst.ins, ScopedClock({None: tick_clock.global_clock})
        )
        si = drain_inst.ins.sync_info
        waits = list(si.on_wait) if si is not None else []
        if len(waits) > 1:
            drain_inst.ins.sync_info = mybir.SyncInfo(
                on_wait=[waits[0]], on_update=list(si.on_update)
            )
            allocated = self.sems.allocated()
            by_name = {}
            for key, sem in allocated.items():
                by_name[str(key)] = sem
                nm = getattr(sem, "name", None)
                if nm is not None:
                    by_name[str(nm)] = sem
            for w in waits[1:]:
                sem = by_name[w.ant_name]
                nc.sync.wait_ge(sem, w.wait_value)
        nc.all_engine_barrier()
        assert self.sems is not None
        popped = nc._tile_sem_poison_stack.pop()
        assert popped is self._sem_poison
        nc.clear_and_free_semaphores(list(self.sems.allocated().values()))
        nc.all_engine_barrier()

    tile.TileContext._drain_and_barrier = _drain_and_barrier
    tile.TileContext._ant_drain_patch = True


def _build_program():
    import concourse.bass as bass
    import concourse.tile as tile
    from concourse import bacc, mybir

    _patch_drain_and_barrier()

    f32 = mybir.dt.float32
    bf16 = mybir.dt.bfloat16
    AF = mybir.ActivationFunctionType
    AFexp = AF.Exp

    # Bacc (not plain Bass): its compile() pipeline runs
    # generate_event_semaphores, which splits multi-sem waits — this
    # walrus build allows at most one sync wait per instruction.
    nc = bacc.Bacc(None)

    xT = nc.dram_tensor("xT", [Dm, S], bf16, kind="ExternalInput")
    Wqkk = nc.dram_tensor("Wqkk", [Dm, QKK], bf16, kind="ExternalInput")
    Wqg = nc.dram_tensor("Wqg", [Dm, DPC], bf16, kind="ExternalInput")
    Wvvg = nc.dram_tensor("Wvvg", [Dm, 2 * DPC], bf16, kind="ExternalInput")
    # per-head bias columns [64, 3]: column h = bias slice for head h
    b_q = nc.dram_tensor("b_q", [HD, HPC], f32, kind="ExternalInput")
    b_k = nc.dram_tensor("b_k", [HD, HPC], f32, kind="ExternalInput")
    b_kg = nc.dram_tensor("b_kg", [HD, HPC], f32, kind="ExternalInput")
    b_qg = nc.dram_tensor("b_qg", [HD, HPC], f32, kind="ExternalInput")
    # broadcast v/vg bias: [128 partitions, head, (v|vg), 64]
    b_vvg = nc.dram_tensor("b_vvg", [128, HPC, 2, HD], f32, kind="ExternalInput")
    out_d = nc.dram_tensor("out", [S, DPC], f32, kind="ExternalOutput")

    packed_np, mask_ops, tcls = _mask_classes()
    NMASK = packed_np.shape[0]
    # device layout [128, nmask, 256]
    masks_d = nc.inline_tensor(
        np.ascontiguousarray(packed_np.transpose(1, 0, 2)).astype(
            ml_dtypes.bfloat16
        ),
        name="masks",
    )

    from contextlib import ExitStack

    with tile.TileContext(nc) as tc, ExitStack() as ctx:
        const = ctx.enter_context(tc.tile_pool(name="const", bufs=1))
        ph = ctx.enter_context(tc.tile_pool(name="ph", bufs=1))
        xpool = ctx.enter_context(tc.tile_pool(name="xpool", bufs=2))
        bx = ctx.enter_context(tc.tile_pool(name="bx", bufs=12))
        sbS = ctx.enter_context(tc.tile_pool(name="sbS", bufs=6))
        psA = ctx.enter_context(tc.tile_pool(name="psA", bufs=3, space="PSUM"))
        psB = ctx.enter_context(tc.tile_pool(name="psB", bufs=2, space="PSUM"))

        # first projection group's operands first (Wqkk, x-tile 0), then
        # everything else — minimizes the startup PE stall
        wqkk = const.tile([128, 6, QKK], bf16, tag="wqkk", name="wqkk")
        nc.sync.dma_start(
            out=wqkk, in_=Wqkk[:, :].rearrange("(c p) d -> p c d", p=128)
        )
        xt0 = xpool.tile([128, 6, 512], bf16, tag="xt", name="xt")
        nc.sync.dma_start(
            out=xt0, in_=xT[:, 0:512].rearrange("(c p) s -> p c s", p=128)
        )

        # ---- remaining constants to SBUF ----
        wvvg = const.tile([128, 6, 2 * DPC], bf16, tag="wvvg", name="wvvg")
        nc.sync.dma_start(
            out=wvvg, in_=Wvvg[:, :].rearrange("(c p) d -> p c d", p=128)
        )
        wqg = const.tile([128, 6, DPC], bf16, tag="wqg", name="wqg")
        nc.sync.dma_start(
            out=wqg, in_=Wqg[:, :].rearrange("(c p) d -> p c d", p=128)
        )
        bias = {}
        for nm, dram in (("q", b_q), ("k", b_k), ("kg", b_kg), ("qg", b_qg)):
            bias[nm] = const.tile([HD, HPC], f32, tag=f"b{nm}", name=f"b{nm}")
            nc.sync.dma_start(out=bias[nm], in_=dram[:])
        bvvg_sb = const.tile([128, HPC, 2, HD], f32, tag="bvvg", name="bvvg_sb")
        nc.sync.dma_start(out=bvvg_sb, in_=b_vvg[:])
        masks_sb = const.tile([128, NMASK, 256], bf16, tag="masks", name="masks_sb")
        nc.sync.dma_start(out=masks_sb, in_=masks_d[:])

        # ---- persistent per-head tensors ----
        qT = [ph.tile([64, S], bf16, tag=f"qT{h}", name=f"qT{h}") for h in range(HPC)]
        kT = [ph.tile([64, S], bf16, tag=f"kT{h}", name=f"kT{h}") for h in range(HPC)]
        kgT = [ph.tile([64, S], bf16, tag=f"kgT{h}", name=f"kgT{h}") for h in range(HPC)]
        # v/vg interleaved with ones column: [:, chunk, 2h+0, :] = v head h,
        # [:, chunk, 2h+1, :] = vg head h ([:, :, :, 64] = 1.0)
        vall = ph.tile([128, NKC, 2 * HPC, HD + 1], bf16, tag="vall", name="vall")
        nc.vector.memset(vall[:, :, :, HD : HD + 1], 1.0)
        selexp = [
            ph.tile([G, S], bf16, tag=f"selexp{h}", name=f"selexp{h}")
            for h in range(HPC)
        ]
        qgT = [ph.tile([64, G], bf16, tag=f"qgT{h}", name=f"qgT{h}") for h in range(HPC)]
        # exp'd global-row scores, transposed: [kpos 128, chunk, head, G]
        eg3 = ph.tile([128, NKC, HPC, G], bf16, tag="eg3", name="eg3")

        def mm(out, lhsT, rhs, start, stop):
            nc.tensor.matmul(out, lhsT, rhs, start=start, stop=stop)

        # (proj, head) destination for merged-column group g = 0..8
        qkk_dst = []
        for g in range(9):
            proj, head = divmod(g, 3)
            qkk_dst.append(
                ([qT, kT, kgT][proj][head], [bias["q"], bias["k"], bias["kg"]][proj], head)
            )

        def emit_proj(st, xt):
            ssl = slice(512 * st, 512 * (st + 1))
            # q/k/kg merged: transposed layout, W stationary
            for dc in range(5):
                d0 = 128 * dc
                d1 = min(d0 + 128, QKK)
                ps = psB.tile([d1 - d0, 512], f32, tag="small", name="psqkk")
                for kc in range(6):
                    mm(ps, wqkk[:, kc, d0:d1], xt[:, kc, :], kc == 0, kc == 5)
                # evacuate per 64-row group, adding bias
                for half in range(2):
                    g0 = d0 + 64 * half
                    if g0 >= QKK:
                        continue
                    dstt, btile, head = qkk_dst[g0 // 64]
                    nc.vector.tensor_scalar_add(
                        dstt[:, ssl],
                        ps[64 * half : 64 * half + 64, :],
                        btile[:, head : head + 1],
                    )

            if st == 0:
                # qg: [64, 16] per head, transposed
                for h in range(HPC):
                    psq = psB.tile([64, G], f32, tag="small", name="psqg")
                    for kc in range(6):
                        mm(
                            psq,
                            wqg[:, kc, HD * h : HD * (h + 1)],
                            xt[:, kc, 0:G],
                            kc == 0,
                            kc == 5,
                        )
                    nc.vector.tensor_scalar_add(qgT[h], psq, bias["qg"][:, h : h + 1])

            # global columns for this s-tile (emitted before v/vg so the
            # Act sel/eg exps drain before the band exps need psA slots):
            # sel = q . k[:G], exp
            for h in range(HPC):
                sps = psB.tile([G, 512], f32, tag="small", name="sps")
                mm(sps, kT[h][:, 0:G], qT[h][:, ssl], True, True)
                nc.scalar.activation(out=selexp[h][:, ssl], in_=sps, func=AFexp)

            # global-row scores for this s-tile's 4 kpos chunks:
            # glT[kpos, g] = kg . qg, then exp -> eg3
            gps = psB.tile([128, 4, HPC, G], f32, tag="small", name="gps")
            for sc in range(4):
                ci = 4 * st + sc
                for h in range(HPC):
                    mm(
                        gps[:, sc, h, :],
                        kgT[h][:, 128 * ci : 128 * (ci + 1)],
                        qgT[h],
                        True,
                        True,
                    )
            nc.scalar.activation(
                out=eg3[:, 4 * st : 4 * st + 4, :, :], in_=gps, func=AFexp
            )

            # v/vg: natural layout, xT chunks stationary
            for sc in range(4):
                ci = 4 * st + sc
                msl = slice(128 * sc, 128 * (sc + 1))
                psv = psB.tile([128, 2 * DPC], f32, tag="small", name="psv")
                for kc in range(6):
                    mm(psv, xt[:, kc, msl], wvvg[:, kc, :], kc == 0, kc == 5)
                # one strided op: psv[:, gi*192 + h*64 + d] -> vall[:, ci, 2h+gi, d]
                src = bass.AP(
                    tensor=psv.tensor,
                    offset=psv.offset,
                    ap=[psv.ap[0], [HD, HPC], [DPC, 2], [1, HD]],
                )
                dst = vall[:, ci, :, 0:HD].rearrange("p (h g) d -> p h g d", h=HPC)
                nc.vector.tensor_add(dst, src, bvvg_sb)

        def emit_qk(t):
            """QK scores (transposed) + exp + mask for one block, 3 heads.

            Scores for chunks [cl, ch) are produced in two [128, 3, 256]
            PSUM tiles (chunk groups 0-2 and 3-5) so the 4-buf psA pool
            recycles with ~3 groups of slack before the PE would wait on
            an Act exp."""
            cl, ch = _chunk_range(t)
            qsl = slice(256 * t, 256 * (t + 1))
            cls = tcls(t)
            bexps = []
            for h in range(HPC):
                bexp = bx.tile([128, 6, 256], bf16, tag="bexp", name="bexp")
                for grp in range(2):
                    g0 = max(cl, 3 * grp)
                    g1 = min(ch, 3 * grp + 3)
                    if g0 >= g1:
                        continue
                    sc_ps = psA.tile([128, 3, 256], f32, tag="scores", name="sc_ps")
                    for c in range(g0, g1):
                        j = 2 * t - 2 + c
                        mm(
                            sc_ps[:, c - 3 * grp, :],
                            kT[h][:, 128 * j : 128 * (j + 1)],
                            qT[h][:, qsl],
                            True,
                            True,
                        )
                    nc.scalar.activation(
                        out=bexp[:, g0:g1, :],
                        in_=sc_ps[:, g0 - 3 * grp : g1 - 3 * grp, :],
                        func=AFexp,
                    )
                for lo, hi, base in mask_ops[cls]:
                    nc.vector.tensor_mul(
                        bexp[:, lo:hi, :],
                        bexp[:, lo:hi, :],
                        masks_sb[:, base : base + (hi - lo), :],
                    )
                bexps.append(bexp)
            return bexps

        def emit_pv(t, bexps):
            """PV + normalize + output DMA for one block, 3 heads."""
            cl, ch = _chunk_range(t)
            for h in range(HPC):
                bexp = bexps[h]
                for half in range(2):
                    q0 = 256 * t + 128 * half
                    hs = slice(128 * half, 128 * (half + 1))
                    at = psB.tile([128, HD + 1], f32, tag="small", name="at")
                    for c in range(cl, ch):
                        j = 2 * t - 2 + c
                        mm(at, bexp[:, c, hs], vall[:, j, 2 * h, :], c == cl, False)
                    mm(
                        at,
                        selexp[h][:, q0 : q0 + 128],
                        vall[0:G, 0, 2 * h, :],
                        False,
                        True,
                    )
                    rec = sbS.tile([128, 1], f32, tag="rec", name="rec")
                    nc.vector.reciprocal(rec, at[:, HD : HD + 1])
                    osb = sbS.tile([128, HD], f32, tag="osb", name="osb")
                    nc.vector.tensor_scalar_mul(osb, at[:, 0:HD], rec)
                    if t == 0 and half == 0:
                        # rows 0..15 are produced by the global-row path at
                        # the tail (disjoint DMA, no race)
                        nc.sync.dma_start(
                            out=out_d[q0 + G : q0 + 128, HD * h : HD * (h + 1)],
                            in_=osb[G:128, :],
                        )
                    else:
                        nc.sync.dma_start(
                            out=out_d[q0 : q0 + 128, HD * h : HD * (h + 1)], in_=osb
                        )

        # ================= fused main loop =================
        prev = []  # (t, bexps) awaiting PV
        for st in range(NST + 1):
            if st < NST:
                if st == 0:
                    xt = xt0
                else:
                    xt = xpool.tile([128, 6, 512], bf16, tag="xt", name="xt")
                    nc.sync.dma_start(
                        out=xt,
                        in_=xT[:, 512 * st : 512 * (st + 1)].rearrange(
                            "(c p) s -> p c s", p=128
                        ),
                    )
                emit_proj(st, xt)
            # QK for blocks whose k-window just completed
            cur = []
            for t in (2 * st - 2, 2 * st - 1):
                if 0 <= t < NB:
                    cur.append((t, emit_qk(t)))
            # PV for the previous iteration's blocks
            for t, bexps in prev:
                emit_pv(t, bexps)
            prev = cur
        for t, bexps in prev:
            emit_pv(t, bexps)

        # ---- global-row tail: PV over all 32 chunks + normalize ----
        for h in range(HPC):
            ops = psB.tile([G, HD + 1], f32, tag="small", name="ops")
            for c in range(NKC):
                mm(ops, eg3[:, c, h, :], vall[:, c, 2 * h + 1, :], c == 0, c == NKC - 1)
            recg = sbS.tile([G, 1], f32, tag="recg", name="recg")
            nc.vector.reciprocal(recg, ops[:, HD : HD + 1])
            outg = sbS.tile([G, HD], f32, tag="outg", name="outg")
            nc.vector.tensor_scalar_mul(outg, ops[:, 0:HD], recg)
            nc.sync.dma_start(out=out_d[0:G, HD * h : HD * (h + 1)], in_=outg)

    return nc


def _get_program():
    if "nc" not in _CACHE:
        nc = _build_program()
        nc.finalize()
        _CACHE["nc"] = nc
    return _CACHE["nc"]


def _prep_in_maps(hidden_states, Wq, bq, Wk, bk, Wv, bv, Wqg, bqg, Wkg, bkg, Wvg, bvg):
    hs = np.asarray(hidden_states, dtype=np.float32)
    f32 = np.float32
    in_maps = []
    for c in range(NCORES):
        b = c // 4
        cols = slice(HD * 3 * (c % 4), HD * (3 * (c % 4) + 3))

        def bcol(v, scale=1.0):
            # [192] -> [64, 3] column-per-head
            return np.ascontiguousarray(
                (np.asarray(v)[cols] * scale).reshape(HPC, HD).T.astype(f32)
            )

        bvvg = np.stack(
            [
                np.asarray(bv)[cols].reshape(HPC, HD),
                np.asarray(bvg)[cols].reshape(HPC, HD),
            ],
            axis=1,
        ).astype(f32)  # [3, 2, 64]
        wqkk = np.concatenate(
            [
                np.asarray(Wq)[:, cols] * SCALE,
                np.asarray(Wk)[:, cols],
                np.asarray(Wkg)[:, cols],
            ],
            axis=1,
        )
        in_maps.append(
            {
                "xT": np.ascontiguousarray(hs[b].T).astype(ml_dtypes.bfloat16),
                "Wqkk": np.ascontiguousarray(wqkk).astype(ml_dtypes.bfloat16),
                "Wqg": np.ascontiguousarray(np.asarray(Wqg)[:, cols] * SCALE).astype(
                    ml_dtypes.bfloat16
                ),
                "Wvvg": np.concatenate(
                    [np.asarray(Wv)[:, cols], np.asarray(Wvg)[:, cols]], axis=1
                ).astype(ml_dtypes.bfloat16),
                "b_q": bcol(bq, SCALE),
                "b_k": bcol(bk),
                "b_kg": bcol(bkg),
                "b_qg": bcol(bqg, SCALE),
                "b_vvg": np.ascontiguousarray(
                    np.broadcast_to(bvvg[None], (128, HPC, 2, HD))
                ),
            }
        )
    return in_maps


def kernel(
    hidden_states,
    Wq,
    bq,
    Wk,
    bk,
    Wv,
    bv,
    Wqg,
    bqg,
    Wkg,
    bkg,
    Wvg,
    bvg,
    n_global,
):
    from concourse.bass_utils import run_bass_kernel_spmd

    assert int(n_global) == G
    nc = _get_program()
    in_maps = _prep_in_maps(
        hidden_states, Wq, bq, Wk, bk, Wv, bv, Wqg, bqg, Wkg, bkg, Wvg, bvg
    )
    res = run_bass_kernel_spmd(nc, in_maps, list(range(NCORES)))
    out = np.zeros((B, S, Dm), np.float32)
    for c in range(NCORES):
        b = c // 4
        cols = slice(HD * 3 * (c % 4), HD * (3 * (c % 4) + 3))
        out[b, :, cols] = res.results[c]["out"]
    return out


# revision 19
# speedup vs baseline: 1.3293x; 1.1548x over previous
"""Longformer self-attention Trainium2 kernel (8-core SPMD), v3.

Sharding: core c handles batch b = c//4 and heads [3*(c%4), 3*(c%4)+3).
Each core computes its 3 heads' [4096, 64] outputs; the host divides by
the softmax denominator (exported as an extra row) and reassembles
[2, 4096, 768].

v3 design notes (on top of v2's fused projection/attention pipeline):
  - PE HAM clock gate: the PE runs at 1.2 GHz unless a ~3.4us activity
    window is busy; small-N matmul stretches re-throttle it.  v3 keeps
    every hot-loop matmul at N>=256:
      * PV is computed transposed: stationary v-chunk [128,65] (64 dims
        + ones column), moving bexp [128 kpos, 256 queries] -> attnT
        [65, 256] in PSUM, one matmul per kpos chunk (no half split).
        Row 64 is the softmax denominator; normalization happens on the
        HOST after the f32 attnT/denominator tile is DMA'd out.
      * QK (K=64 contraction) issues as concurrent row-tile pairs:
        heads 0/1 are packed at partitions 0-63/64-127 of shared q/k
        tiles, head 2 is duplicated into both halves via SBUF DMA and
        pairs its own chunks.  tile_position=(0,0)/(64,0); the pair's
        PSUM outputs come from disjoint even/odd pools (bank safety).
      * global-row PV col-tiles 3 heads at tile_position (0,0/32/64).
  - Weight columns are reordered per-core so each 128-wide PSUM
    d-chunk evacuates with a single DVE add: [q0|q1][k0|k1][q2|k2]
    [kg0|kg1][kg2].  Biases ride along as per-partition columns.
  - Masks: multiplicative 0/1 bf16 masks after exp (packed classes).
  - exp() without max subtraction (logits are O(1), safe in f32).
"""

import sys

sys.path.insert(0, "/opt/trn_rl_repo")

import numpy as np
import ml_dtypes

B, S, Dm, H, WIN, G, HD = 2, 4096, 768, 12, 256, 16, 64
HPC = 3            # heads per core
NCORES = 8
DPC = HPC * HD     # 192 output dims per core
NB = S // WIN      # 16 query blocks
NKC = S // 128     # 32 kpos chunks of 128
NST = 8            # s-tiles of 512
SCALE = 1.0 / 8.0  # 1/sqrt(64)
QKK = 3 * DPC      # 576 merged q|k|kg output cols

_CACHE = {}


def _chunk_range(t):
    if t == 0:
        return 2, 6
    if t == NB - 1:
        return 0, 4
    return 0, 6


def _build_mask(t, c):
    p = np.arange(128)[:, None]
    r = np.arange(256)[None, :]
    kpos = (2 * t - 2 + c) * 128 + p
    i = 256 * t + r
    keep = (np.abs(kpos - i) <= WIN) & (kpos >= 0) & (kpos < S) & (kpos >= G)
    return keep.astype(np.float32)


def _mask_classes():
    """Packed mask chunks [nmask, 128, 256] and, per class, the list of
    (chunk_lo, chunk_hi, packed_idx) multiply ops."""
    packed = []

    def cls_of(t):
        return 0 if t == 0 else (1 if t == 1 else (3 if t == NB - 1 else 2))

    ops = {}
    for cls, trep in ((0, 0), (1, 1), (2, 7), (3, NB - 1)):
        cl, ch = _chunk_range(trep)
        need = [
            c for c in range(cl, ch) if not np.all(_build_mask(trep, c) == 1.0)
        ]
        groups = []
        for c in need:
            if groups and groups[-1][-1] == c - 1:
                groups[-1].append(c)
            else:
                groups.append([c])
        clsops = []
        for grp in groups:
            base = len(packed)
            for c in grp:
                packed.append(_build_mask(trep, c))
            clsops.append((grp[0], grp[-1] + 1, base))
        ops[cls] = clsops
    for t in range(NB):
        cls = cls_of(t)
        cl, ch = _chunk_range(t)
        covered = set()
        for lo, hi, base in ops[cls]:
            for c in range(lo, hi):
                assert np.array_equal(packed[base + (c - lo)], _build_mask(t, c))
                covered.add(c)
        for c in range(cl, ch):
            if c not in covered:
                assert np.all(_build_mask(t, c) == 1.0), (t, c)
    return np.stack(packed), ops, cls_of


def _patch_drain_and_barrier():
    """The walrus build in this container rejects >1 sync-wait on the CTRL
    (Drain) instruction that TileContext emits at exit.  Split the waits:
    keep one on the drain, emit the rest as explicit single-sem wait_ge
    instructions on the sync engine before the barrier."""
    import concourse.tile as tile
    from concourse import mybir
    from concourse.vector_clock import ScopedClock

    if getattr(tile.TileContext, "_ant_drain_patch", False):
        return

    def _drain_and_barrier(self, tick_clock, wait_clock):
        nc = self.nc
        drain_inst = nc.sync.drain()
        wait_clock.add_sem_waits(
            drain_inst.ins, ScopedClock({None: tick_clock.global_clock})
        )
        si = drain_inst.ins.sync_info
        waits = list(si.on_wait) if si is not None else []
        if len(waits) > 1:
            drain_inst.ins.sync_info = mybir.SyncInfo(
                on_wait=[waits[0]], on_update=list(si.on_update)
            )
            allocated = self.sems.allocated()
            by_name = {}
            for key, sem in allocated.items():
                by_name[str(key)] = sem
                nm = getattr(sem, "name", None)
                if nm is not None:
                    by_name[str(nm)] = sem
            for w in waits[1:]:
                sem = by_name[w.ant_name]
                nc.sync.wait_ge(sem, w.wait_value)
        nc.all_engine_barrier()
        assert self.sems is not None
        popped = nc._tile_sem_poison_stack.pop()
        assert popped is self._sem_poison
        nc.clear_and_free_semaphores(list(self.sems.allocated().values()))
        nc.all_engine_barrier()

    tile.TileContext._drain_and_barrier = _drain_and_barrier
    tile.TileContext._ant_drain_patch = True


def _build_program():
    import concourse.bass as bass
    import concourse.tile as tile
    from concourse import bacc, mybir

    _patch_drain_and_barrier()

    f32 = mybir.dt.float32
    bf16 = mybir.dt.bfloat16
    AF = mybir.ActivationFunctionType
    AFexp = AF.Exp

    nc = bacc.Bacc(None)

    xT = nc.dram_tensor("xT", [Dm, S], bf16, kind="ExternalInput")
    # merged+reordered weight: [q0|q1][k0|k1][q2|k2][kg0|kg1][kg2]
    Wqkk = nc.dram_tensor("Wqkk", [Dm, QKK], bf16, kind="ExternalInput")
    # qg weight reordered [qg0|qg1][qg2]
    Wqg = nc.dram_tensor("Wqg", [Dm, DPC], bf16, kind="ExternalInput")
    Wvvg = nc.dram_tensor("Wvvg", [Dm, 2 * DPC], bf16, kind="ExternalInput")
    # bias columns [128, 8]: col dc<5 = qkk evac bias for that d-chunk
    # (stacked per 64-row half); col 5 rows 0-63 = bk_h2; col 6 = qg01
    # stacked; col 7 rows 0-63 = bqg_h2
    b_all = nc.dram_tensor("b_all", [128, 8], f32, kind="ExternalInput")
    b_vvg = nc.dram_tensor("b_vvg", [128, HPC, 2, HD], f32, kind="ExternalInput")
    # unnormalized attn output (row 64 = softmax denominator)
    outT_d = nc.dram_tensor("outT", [HPC, HD + 1, S], f32, kind="ExternalOutput")
    outG_d = nc.dram_tensor("outG", [96, HD + 1], f32, kind="ExternalOutput")

    packed_np, mask_ops, tcls = _mask_classes()
    NMASK = packed_np.shape[0]
    masks_d = nc.inline_tensor(
        np.ascontiguousarray(packed_np.transpose(1, 0, 2)).astype(
            ml_dtypes.bfloat16
        ),
        name="masks",
    )

    from contextlib import ExitStack

    with tile.TileContext(nc) as tc, ExitStack() as ctx:
        const = ctx.enter_context(tc.tile_pool(name="const", bufs=1))
        ph = ctx.enter_context(tc.tile_pool(name="ph", bufs=1))
        xpool = ctx.enter_context(tc.tile_pool(name="xpool", bufs=2))
        bx = ctx.enter_context(tc.tile_pool(name="bx", bufs=12))
        ob = ctx.enter_context(tc.tile_pool(name="ob", bufs=4))
        # QK score pools: pair partners must land in different PSUM banks,
        # so base-0 tiles come from psAe and base-64 tiles from psAo.
        psAe = ctx.enter_context(tc.tile_pool(name="psAe", bufs=3, space="PSUM"))
        psAo = ctx.enter_context(tc.tile_pool(name="psAo", bufs=3, space="PSUM"))
        psB = ctx.enter_context(tc.tile_pool(name="psB", bufs=2, space="PSUM"))

        wqkk = const.tile([128, 6, QKK], bf16, tag="wqkk", name="wqkk")
        nc.sync.dma_start(
            out=wqkk, in_=Wqkk[:, :].rearrange("(c p) d -> p c d", p=128)
        )
        xt0 = xpool.tile([128, 6, 512], bf16, tag="xt", name="xt")
        nc.sync.dma_start(
            out=xt0, in_=xT[:, 0:512].rearrange("(c p) s -> p c s", p=128)
        )

        wvvg = const.tile([128, 6, 2 * DPC], bf16, tag="wvvg", name="wvvg")
        nc.sync.dma_start(
            out=wvvg, in_=Wvvg[:, :].rearrange("(c p) d -> p c d", p=128)
        )
        wqg = const.tile([128, 6, DPC], bf16, tag="wqg", name="wqg")
        nc.sync.dma_start(
            out=wqg, in_=Wqg[:, :].rearrange("(c p) d -> p c d", p=128)
        )
        ball = const.tile([128, 8], f32, tag="ball", name="ball")
        nc.sync.dma_start(out=ball, in_=b_all[:])
        bvvg_sb = const.tile([128, HPC, 2, HD], f32, tag="bvvg", name="bvvg_sb")
        nc.sync.dma_start(out=bvvg_sb, in_=b_vvg[:])
        masks_sb = const.tile([128, NMASK, 256], bf16, tag="masks", name="masks_sb")
        nc.sync.dma_start(out=masks_sb, in_=masks_d[:])

        # ---- persistent tensors ----
        # packed transposed projections: [0:64]=head A, [64:128]=head B
        qP = ph.tile([128, S], bf16, tag="qP", name="qP")     # q0|q1
        kP = ph.tile([128, S], bf16, tag="kP", name="kP")     # k0|k1
        kgP = ph.tile([128, S], bf16, tag="kgP", name="kgP")  # kg0|kg1
        q2 = ph.tile([128, S], bf16, tag="q2", name="q2")     # q2|q2(dup)
        k2 = ph.tile([128, S], bf16, tag="k2", name="k2")     # k2|k2(dup)
        kg2 = ph.tile([128, S], bf16, tag="kg2", name="kg2")  # kg2|kg2(dup)
        qgP = ph.tile([128, G], bf16, tag="qgP", name="qgP")  # qg0|qg1
        qg2 = ph.tile([128, G], bf16, tag="qg2", name="qg2")  # qg2|qg2(dup)

        def qhalf(h):  # (tile, part_lo) for q of head h
            return (qP, 0) if h == 0 else ((qP, 64) if h == 1 else (q2, 0))

        def khalf(h):
            return (kP, 0) if h == 0 else ((kP, 64) if h == 1 else (k2, 0))

        # v/vg interleaved with ones column: [:, chunk, 2h+0, :] = v head h,
        # [:, chunk, 2h+1, :] = vg head h ([:, :, :, 64] = 1.0)
        vall = ph.tile([128, NKC, 2 * HPC, HD + 1], bf16, tag="vall", name="vall")
        nc.vector.memset(vall[:, :, :, HD : HD + 1], 1.0)
        selexp = [
            ph.tile([G, S], bf16, tag=f"selexp{h}", name=f"selexp{h}")
            for h in range(HPC)
        ]
        eg3 = ph.tile([128, NKC, HPC, G], bf16, tag="eg3", name="eg3")

        def mm(out, lhsT, rhs, start, stop, tile_position=None):
            nc.tensor.matmul(
                out, lhsT, rhs, start=start, stop=stop, tile_position=tile_position
            )

        # qkk evac destinations per d-chunk: (dst_tile, rows, bias_col_ap)
        def emit_proj(st, xt):
            ssl = slice(512 * st, 512 * (st + 1))
            evac = [
                [(qP, None, 0)],
                [(kP, None, 1)],
                [(q2, (0, 64), 2), (k2, (64, 128), 5)],
                [(kgP, None, 3)],
                [(kg2, (0, 64), 4)],
            ]
            for dc in range(5):
                d0 = 128 * dc
                d1 = min(d0 + 128, QKK)
                ps = psB.tile([d1 - d0, 512], f32, tag="small", name="psqkk")
                for kc in range(6):
                    mm(ps, wqkk[:, kc, d0:d1], xt[:, kc, :], kc == 0, kc == 5)
                for dst, rows, bcol in evac[dc]:
                    if rows is None:
                        nc.vector.tensor_scalar_add(
                            dst[0 : d1 - d0, ssl], ps, ball[0 : d1 - d0, bcol : bcol + 1]
                        )
                    else:
                        r0, r1 = rows
                        nc.vector.tensor_scalar_add(
                            dst[0 : r1 - r0, ssl],
                            ps[r0:r1, :],
                            ball[0 : r1 - r0, bcol : bcol + 1],
                        )
            # duplicate head-2 halves: rows 0-63 -> rows 64-127 (QK pairing)
            for dst in (q2, k2):
                nc.sync.dma_start(out=dst[64:128, ssl], in_=dst[0:64, ssl])

            if st == 0:
                # qg: [128, 16] = qg0|qg1 stacked (full-M matmul), + qg2
                psq = psB.tile([128, G], f32, tag="small", name="psqg")
                for kc in range(6):
                    mm(psq, wqg[:, kc, 0:128], xt[:, kc, 0:G], kc == 0, kc == 5)
                nc.vector.tensor_scalar_add(qgP, psq, ball[:, 6:7])
                psq2 = psB.tile([64, G], f32, tag="small", name="psqg2")
                for kc in range(6):
                    mm(psq2, wqg[:, kc, 128:192], xt[:, kc, 0:G], kc == 0, kc == 5)
                nc.vector.tensor_scalar_add(qg2[0:64, :], psq2, ball[0:64, 7:8])

            # sel = q . k[:G] for this s-tile's queries; heads 0/1 paired
            sel01 = []
            for h in range(2):
                kt, p0 = khalf(h)
                qt, _ = qhalf(h)
                sps = psB.tile([G, 512], f32, tag="small", name="sps")
                mm(
                    sps,
                    kt[p0 : p0 + 64, 0:G],
                    qt[p0 : p0 + 64, ssl],
                    True,
                    True,
                    tile_position=(p0, 0),
                )
                sel01.append(sps)
            for h in range(2):
                nc.scalar.activation(
                    out=selexp[h][:, ssl], in_=sel01[h], func=AFexp
                )
            sps2 = psB.tile([G, 512], f32, tag="small", name="sps")
            mm(sps2, k2[0:64, 0:G], q2[0:64, ssl], True, True)
            nc.scalar.activation(out=selexp[2][:, ssl], in_=sps2, func=AFexp)

            # global-row scores for this s-tile's 4 kpos chunks; head-outer
            # with exp emitted per head so psB pool recycling never waits
            # on a consumer that sits later in the PE queue.
            for h, (kgt, p0) in enumerate(((kgP, 0), (kgP, 64), (kg2, 0))):
                qgt = qgP if h < 2 else qg2
                gps = psB.tile([128, 4, G], f32, tag="small", name="gps")
                for sc in range(4):
                    ci = 4 * st + sc
                    csl = slice(128 * ci, 128 * (ci + 1))
                    mm(
                        gps[:, sc, :],
                        kgt[p0 : p0 + 64, csl],
                        qgt[p0 : p0 + 64, :],
                        True,
                        True,
                        tile_position=(p0, 0),
                    )
                nc.scalar.activation(
                    out=eg3[:, 4 * st : 4 * st + 4, h, :], in_=gps, func=AFexp
                )

            # v/vg: natural layout, xT chunks stationary
            for sc in range(4):
                ci = 4 * st + sc
                msl = slice(128 * sc, 128 * (sc + 1))
                psv = psB.tile([128, 2 * DPC], f32, tag="small", name="psv")
                for kc in range(6):
                    mm(psv, xt[:, kc, msl], wvvg[:, kc, :], kc == 0, kc == 5)
                src = bass.AP(
                    tensor=psv.tensor,
                    offset=psv.offset,
                    ap=[psv.ap[0], [HD, HPC], [DPC, 2], [1, HD]],
                )
                dst = vall[:, ci, :, 0:HD].rearrange("p (h g) d -> p h g d", h=HPC)
                nc.vector.tensor_add(dst, src, bvvg_sb)

        def emit_qk(t):
            """Paired QK scores + exp + mask for one block, 3 heads."""
            cl, ch = _chunk_range(t)
            n = ch - cl
            qsl = slice(256 * t, 256 * (t + 1))
            cls = tcls(t)
            bexps = [
                bx.tile([128, 6, 256], bf16, tag="bexp", name="bexp")
                for _ in range(HPC)
            ]

            # heads 0/1 pair on the same chunk (rows 0-63 vs 64-127); head 2
            # pairs chunk cl+i (base 0) with chunk cl+n/2+i (base 64, dup).
            # New [128,2,256] tile every 2 chunks.
            tiles01 = {0: [], 1: []}
            for ci, c in enumerate(range(cl, ch)):
                j = 2 * t - 2 + c
                jsl = slice(128 * j, 128 * (j + 1))
                if ci % 2 == 0:
                    se = psAe.tile([128, 2, 256], f32, tag="se", name="se")
                    so = psAo.tile([128, 2, 256], f32, tag="so", name="so")
                    tiles01[0].append((se, c))
                    tiles01[1].append((so, c))
                slot = ci % 2
                mm(se[:, slot, :], kP[0:64, jsl], qP[0:64, qsl], True, True,
                   tile_position=(0, 0))
                mm(so[:, slot, :], kP[64:128, jsl], qP[64:128, qsl], True, True,
                   tile_position=(64, 0))
                if slot == 1 or ci == n - 1:
                    width = slot + 1
                    for h, tl in ((0, tiles01[0][-1]), (1, tiles01[1][-1])):
                        tile_, c0 = tl
                        nc.scalar.activation(
                            out=bexps[h][:, c0 : c0 + width, :],
                            in_=tile_[:, 0:width, :],
                            func=AFexp,
                        )
            # head 2: pair (cl+i, cl+n/2+i) via the duplicated halves
            half = n // 2
            t2e, t2o = [], []
            for i in range(half):
                ca = cl + i
                cb = cl + half + i
                ja = 2 * t - 2 + ca
                jb = 2 * t - 2 + cb
                if i % 2 == 0:
                    se = psAe.tile([128, 2, 256], f32, tag="se", name="se")
                    so = psAo.tile([128, 2, 256], f32, tag="so", name="so")
                    t2e.append((se, ca))
                    t2o.append((so, cb))
                slot = i % 2
                mm(se[:, slot, :], k2[0:64, 128 * ja : 128 * ja + 128],
                   q2[0:64, qsl], True, True, tile_position=(0, 0))
                mm(so[:, slot, :], k2[64:128, 128 * jb : 128 * jb + 128],
                   q2[64:128, qsl], True, True, tile_position=(64, 0))
                if slot == 1 or i == half - 1:
                    width = slot + 1
                    for tile_, c0 in (t2e[-1], t2o[-1]):
                        nc.scalar.activation(
                            out=bexps[2][:, c0 : c0 + width, :],
                            in_=tile_[:, 0:width, :],
                            func=AFexp,
                        )
            for h in range(HPC):
                for lo, hi, base in mask_ops[cls]:
                    nc.vector.tensor_mul(
                        bexps[h][:, lo:hi, :],
                        bexps[h][:, lo:hi, :],
                        masks_sb[:, base : base + (hi - lo), :],
                    )
            return bexps

        def emit_pv(t, bexps):
            """Transposed PV: attnT [65, 256] per head; DMA out unnormalized."""
            cl, ch = _chunk_range(t)
            qsl = slice(256 * t, 256 * (t + 1))
            for h in range(HPC):
                at = psB.tile([HD + 1, 256], f32, tag="small", name="at")
                for c in range(cl, ch):
                    j = 2 * t - 2 + c
                    mm(at, vall[:, j, 2 * h, :], bexps[h][:, c, :], c == cl, False)
                mm(at, vall[0:G, 0, 2 * h, :], selexp[h][:, qsl], False, True)
                osb = ob.tile([HD + 1, 256], f32, tag="osb", name="osb")
                nc.vector.tensor_copy(out=osb, in_=at)
                nc.sync.dma_start(out=outT_d[h, :, qsl], in_=osb)

        # ================= fused main loop =================
        prev = []
        for st in range(NST + 1):
            if st < NST:
                if st == 0:
                    xt = xt0
                else:
                    xt = xpool.tile([128, 6, 512], bf16, tag="xt", name="xt")
                    nc.sync.dma_start(
                        out=xt,
                        in_=xT[:, 512 * st : 512 * (st + 1)].rearrange(
                            "(c p) s -> p c s", p=128
                        ),
                    )
                emit_proj(st, xt)
            cur = []
            for t in (2 * st - 2, 2 * st - 1):
                if 0 <= t < NB:
                    cur.append((t, emit_qk(t)))
            for t, bexps in prev:
                emit_pv(t, bexps)
            prev = cur
        for t, bexps in prev:
            emit_pv(t, bexps)

        # ---- global-row tail: col-tiled PV over all 32 chunks ----
        ops3 = psB.tile([96, HD + 1], f32, tag="small", name="ops3")
        for c in range(NKC):
            for h in range(HPC):
                mm(
                    ops3[32 * h : 32 * h + G, :],
                    eg3[:, c, h, :],
                    vall[:, c, 2 * h + 1, :],
                    c == 0,
                    c == NKC - 1,
                    tile_position=(0, 32 * h),
                )
        og = ob.tile([96, HD + 1], f32, tag="og", name="og")
        nc.vector.tensor_copy(out=og, in_=ops3)
        nc.sync.dma_start(out=outG_d[:], in_=og)

    return nc


def _get_program():
    if "nc" not in _CACHE:
        nc = _build_program()
        nc.finalize()
        _CACHE["nc"] = nc
    return _CACHE["nc"]


def _prep_in_maps(hidden_states, Wq, bq, Wk, bk, Wv, bv, Wqg, bqg, Wkg, bkg, Wvg, bvg):
    hs = np.asarray(hidden_states, dtype=np.float32)
    f32 = np.float32
    in_maps = []
    for c in range(NCORES):
        b = c // 4
        cols = slice(HD * 3 * (c % 4), HD * (3 * (c % 4) + 3))

        wq = np.asarray(Wq)[:, cols] * SCALE     # [768, 192]
        wk = np.asarray(Wk)[:, cols]
        wkg = np.asarray(Wkg)[:, cols]
        wqgc = np.asarray(Wqg)[:, cols] * SCALE
        hcols = [slice(HD * h, HD * (h + 1)) for h in range(HPC)]
        # [q0|q1][k0|k1][q2|k2][kg0|kg1][kg2]
        wqkk = np.concatenate(
            [
                wq[:, hcols[0]], wq[:, hcols[1]],
                wk[:, hcols[0]], wk[:, hcols[1]],
                wq[:, hcols[2]], wk[:, hcols[2]],
                wkg[:, hcols[0]], wkg[:, hcols[1]],
                wkg[:, hcols[2]],
            ],
            axis=1,
        )
        wqgr = np.concatenate(
            [wqgc[:, hcols[0]], wqgc[:, hcols[1]], wqgc[:, hcols[2]]], axis=1
        )

        def seg(v, h, scale=1.0):
            return (np.asarray(v)[cols][HD * h : HD * (h + 1)] * scale).astype(f32)

        ball = np.zeros((128, 8), f32)
        ball[:, 0] = np.concatenate([seg(bq, 0, SCALE), seg(bq, 1, SCALE)])
        ball[:, 1] = np.concatenate([seg(bk, 0), seg(bk, 1)])
        ball[:, 2] = np.concatenate([seg(bq, 2, SCALE), np.zeros(64, f32)])
        ball[:, 3] = np.concatenate([seg(bkg, 0), seg(bkg, 1)])
        ball[:, 4] = np.concatenate([seg(bkg, 2), np.zeros(64, f32)])
        ball[:, 5] = np.concatenate([seg(bk, 2), np.zeros(64, f32)])
        ball[:, 6] = np.concatenate([seg(bqg, 0, SCALE), seg(bqg, 1, SCALE)])
        ball[:, 7] = np.concatenate([seg(bqg, 2, SCALE), np.zeros(64, f32)])

        bvvg = np.stack(
            [
                np.asarray(bv)[cols].reshape(HPC, HD),
                np.asarray(bvg)[cols].reshape(HPC, HD),
            ],
            axis=1,
        ).astype(f32)
        in_maps.append(
            {
                "xT": np.ascontiguousarray(hs[b].T).astype(ml_dtypes.bfloat16),
                "Wqkk": np.ascontiguousarray(wqkk).astype(ml_dtypes.bfloat16),
                "Wqg": np.ascontiguousarray(wqgr).astype(ml_dtypes.bfloat16),
                "Wvvg": np.concatenate(
                    [np.asarray(Wv)[:, cols], np.asarray(Wvg)[:, cols]], axis=1
                ).astype(ml_dtypes.bfloat16),
                "b_all": ball,
                "b_vvg": np.ascontiguousarray(
                    np.broadcast_to(bvvg[None], (128, HPC, 2, HD))
                ),
            }
        )
    return in_maps


def kernel(
    hidden_states,
    Wq,
    bq,
    Wk,
    bk,
    Wv,
    bv,
    Wqg,
    bqg,
    Wkg,
    bkg,
    Wvg,
    bvg,
    n_global,
):
    from concourse.bass_utils import run_bass_kernel_spmd

    assert int(n_global) == G
    nc = _get_program()
    in_maps = _prep_in_maps(
        hidden_states, Wq, bq, Wk, bk, Wv, bv, Wqg, bqg, Wkg, bkg, Wvg, bvg
    )
    res = run_bass_kernel_spmd(nc, in_maps, list(range(NCORES)))
    out = np.zeros((B, S, Dm), np.float32)
    for c in range(NCORES):
        b = c // 4
        base = HD * 3 * (c % 4)
        outT = res.results[c]["outT"]  # [3, 65, 4096]
        outG = res.results[c]["outG"]  # [96, 65]
        for h in range(HPC):
            att = outT[h, 0:HD, :] / outT[h, HD : HD + 1, :]
            out[b, :, base + HD * h : base + HD * (h + 1)] = att.T
            og = outG[32 * h : 32 * h + G, 0:HD] / outG[32 * h : 32 * h + G, HD:]
            out[b, 0:G, base + HD * h : base + HD * (h + 1)] = og
    return out
